# revision 2
# baseline (speedup 1.0000x reference)
"""Distributed LightGCN propagation on 8 TRN2 NeuronCores - DMA-gather edition.

Per core (SPMD on 8 cores), per graph, per layer:
- Table lives in HBM as f32 [npad, 64] (256B rows). Node space is permuted
  block-cyclically so core k's dst shard is rows [k*nc_rows, (k+1)*nc_rows).
- Edges are owned by the core of their dst block, sorted by
  (src window of 32768 rows, dst block), each (window, block) bucket padded
  to the 128-multiple of the max count over cores (SPMD-uniform stream).
- Per gather call (8 tiles = 1024 edges; >=2048 wedges the SWDGE ring):
  gpsimd.dma_gather pulls src rows straight from HBM into edge-major SBUF
  [128, 8, 64] f32 (1 descriptor per edge, 256B each); DVE scales by edge
  weight with a bf16 cast; DVE builds one-hot S from dst lanes; TensorE
  accumulates S^T @ msgs into the dst block's PSUM; DVE adds finished
  blocks into cur (SBUF, f32). Meta (idx/val/dst) is DMA'd in groups of
  ~128 tiles to amortize issue cost.
- Row L2 norms + acc as in the reference; acc parked in DRAM.
- Between layers: cur -> f32 shard rows in DRAM -> AllGather into a
  Shared HBM table for layer 2's gathers.
"""

import math
import sys

sys.path.insert(0, "/opt/trn_rl_repo")

import numpy as np
import ml_dtypes

import concourse.mybir as mybir
import concourse.tile as tile
from concourse import bacc
from concourse.bass_utils import run_bass_kernel_spmd

D = 64
NCORES = 8
W = 32768        # gather window rows (int16 idx)
CT = 8           # tiles (of 128 edges) per dma_gather call
MCT = 128        # meta-group size in tiles
NU, NBU, NI_ = 100000, 20000, 50000
F32 = mybir.dt.float32
BF16 = mybir.dt.bfloat16
I16 = mybir.dt.int16

LAST_EXEC_NS = None


def _roundup(x, m):
    return (x + m - 1) // m * m


class GMeta:
    def __init__(self, name, rows, cols, vals, n, n_cores=NCORES):
        self.name = name
        self.n = n
        NBG = math.ceil(n / 128)
        self.NB = NB = math.ceil(NBG / n_cores)
        self.nc_rows = NB * 128
        self.npad = n_cores * self.nc_rows
        self.NW = NW = math.ceil(self.npad / W)
        self.wlen = [min(W, self.npad - w * W) for w in range(NW)]

        def perm(r):
            j = r // 128
            return (j % n_cores) * self.nc_rows + (j // n_cores) * 128 + r % 128

        self._perm = perm

        rows = np.asarray(rows).astype(np.int64)
        cols = np.asarray(cols).astype(np.int64)
        vals = np.asarray(vals).astype(np.float32)
        jb = rows // 128
        k = jb % n_cores
        B = jb // n_cores
        lane = rows % 128
        pcol = perm(cols)
        w = pcol // W
        srel = pcol % W

        key = (k * NW + w) * NB + B
        L = np.bincount(key, minlength=n_cores * NW * NB).reshape(n_cores, NW, NB)
        P = np.where(L.max(axis=0) > 0, _roundup(L.max(axis=0), 128), 0)

        # tile lists per window; gell0 = global tile index of bucket start
        self.tl = []          # per window: list of (B, first, last)
        gell0 = np.full((NW, NB), -1, np.int64)
        tau = 0
        self.tau0 = []
        for ww in range(NW):
            self.tau0.append(tau)
            lst = []
            for BB in range(NB):
                if P[ww, BB] > 0:
                    gell0[ww, BB] = tau + len(lst)
                    lst += [BB] * (P[ww, BB] // 128)
            flg = []
            for i, BB in enumerate(lst):
                first = i == 0 or lst[i - 1] != BB
                last = i == len(lst) - 1 or lst[i + 1] != BB
                flg.append((BB, first, last))
            self.tl.append(flg)
            tau += len(lst)
        self.Ttot = tau
        # calls: (window, global tile t0, local tile r0, ntiles)
        self.calls = []
        for ww in range(NW):
            r = 0
            nt_w = len(self.tl[ww])
            while r < nt_w:
                nt = min(CT, nt_w - r)
                self.calls.append((ww, self.tau0[ww] + r, r, nt))
                r += nt
        # meta groups: consecutive calls spanning <= MCT tiles:
        # (T0, Tn, first call idx, ncalls)
        self.groups = []
        ci = 0
        while ci < len(self.calls):
            T0 = self.calls[ci][1]
            cj = ci
            Tn = 0
            while cj < len(self.calls):
                _, t0, _, nt = self.calls[cj]
                if t0 + nt - T0 > MCT:
                    break
                Tn = t0 + nt - T0
                cj += 1
            self.groups.append((T0, Tn, ci, cj - ci))
            ci = cj
        self.call2grp = np.zeros(len(self.calls), np.int64)
        for gidx, (_, _, c0, ncl) in enumerate(self.groups):
            self.call2grp[c0 : c0 + ncl] = gidx

        # per-core arrays
        self.idxw, self.val, self.dst = [], [], []
        for kk in range(n_cores):
            sel = k == kk
            ws, Bs = w[sel], B[sel]
            srels, lds, vv = srel[sel], lane[sel], vals[sel]
            okey = ws * NB + Bs
            order = np.argsort(okey, kind="stable")
            skey = okey[order]
            first = np.concatenate([[True], skey[1:] != skey[:-1]])
            run_id = np.cumsum(first) - 1
            run_start = np.concatenate([[0], np.nonzero(first)[0][1:]])
            rank_sorted = np.arange(len(skey)) - run_start[run_id]
            rank = np.empty_like(rank_sorted)
            rank[order] = rank_sorted
            e = 128 * gell0[ws, Bs] + rank
            idx_arr = np.zeros((128, 8 * self.Ttot), np.int16)
            val_arr = np.zeros((128, self.Ttot), np.float32)
            dst_arr = np.zeros((128, self.Ttot), np.float32)
            for g in range(8):
                idx_arr[16 * g + e % 16, e // 16] = srels.astype(np.int16)
            val_arr[e % 128, e // 128] = vv
            dst_arr[e % 128, e // 128] = lds.astype(np.float32)
            self.idxw.append(idx_arr)
            self.val.append(val_arr)
            self.dst.append(dst_arr.astype(ml_dtypes.bfloat16))

    def ptable(self, table):
        pt = np.zeros((self.npad, D), np.float32)
        pt[self._perm(np.arange(self.n))] = table
        return np.ascontiguousarray(pt)

    def shard0(self, ptab):
        out = []
        for kk in range(NCORES):
            sh = ptab[kk * self.nc_rows : (kk + 1) * self.nc_rows]
            out.append(
                np.ascontiguousarray(
                    sh.reshape(self.NB, 128, D).transpose(1, 0, 2)
                ).reshape(128, self.NB * D)
            )
        return out

    def unshard(self, outs):
        parts = []
        for kk in range(NCORES):
            a = outs[kk].reshape(128, self.NB, D).transpose(1, 0, 2)
            parts.append(a.reshape(self.NB * 128, D))
        return np.concatenate(parts, axis=0)[self._perm(np.arange(self.n))]


def _consts():
    iota8 = np.tile(np.arange(128, dtype=np.float32), (128, 8))
    return {"iota8": iota8.astype(ml_dtypes.bfloat16)}


def _spmm_layer(nc, m, g, layer, src_tab, p_idx, p_val, p_dst, consts_sb,
                pools, cur_sb, pending=None):
    """Emit one graph-layer: gather-call loop -> S build -> scatter matmuls."""
    mTp, metap, msgp, sp, pp = (
        pools["mTp"], pools["metap"], pools["msgp"], pools["sp"], pools["pp"],
    )
    iota8 = consts_sb["iota8"]
    iota3 = iota8.rearrange("p (t j) -> p t j", j=128)
    cur3 = cur_sb.rearrange("p (b d) -> p b d", d=D)
    nc.vector.memset(cur_sb[:, : m.NB * D], 0.0)
    psB = {}
    meta_tiles = {}

    def issue_meta(gi):
        T0, Tn, _, _ = m.groups[gi]
        idx_sb = metap.tile([128, 8 * MCT], I16, tag="idx", name="idx")
        val_sb = metap.tile([128, MCT], F32, tag="val", name="val")
        dst_sb = metap.tile([128, MCT], BF16, tag="dst", name="dst")
        nc.scalar.dma_start(out=idx_sb[:, : 8 * Tn], in_=p_idx[:, 8 * T0 : 8 * (T0 + Tn)])
        nc.sync.dma_start(out=val_sb[:, :Tn], in_=p_val[:, T0 : T0 + Tn])
        nc.sync.dma_start(out=dst_sb[:, :Tn], in_=p_dst[:, T0 : T0 + Tn])
        meta_tiles[gi] = (idx_sb, val_sb, dst_sb)

    for gi in range(min(2, len(m.groups))):
        issue_meta(gi)
    cur_grp = -1
    grp_sb = None
    for ci, (ww, t0, r0, nt) in enumerate(m.calls):
        gi = int(m.call2grp[ci])
        if gi != cur_grp:
            if cur_grp >= 0:
                meta_tiles.pop(cur_grp)
            if gi + 2 < len(m.groups):
                issue_meta(gi + 2)
            grp_sb = meta_tiles[gi]
            cur_grp = gi
        idx_sb, val_sb, dst_sb = grp_sb
        o = t0 - m.groups[gi][0]
        mT = mTp.tile([128, CT * D], F32, tag="mT", name="mT")
        mT3 = mT.rearrange("p (t f) -> p t f", f=D)
        nc.gpsimd.dma_gather(
            mT3[:, :nt, :],
            src_tab[ww * W : ww * W + m.wlen[ww], :],
            idx_sb[:, 8 * o : 8 * (o + nt)],
            128 * nt,
            128 * nt,
            D,
            queue_num=ci % 4,
        )
        msgs = msgp.tile([128, CT * D], BF16, tag="msgs", name="msgs")
        msgs3 = msgs.rearrange("p (t f) -> p t f", f=D)
        nc.vector.tensor_tensor(
            out=msgs3[:, :nt, :],
            in0=mT3[:, :nt, :],
            in1=val_sb[:, o : o + nt].unsqueeze(2).to_broadcast([128, nt, D]),
            op=mybir.AluOpType.mult,
        )
        S_r = sp.tile([128, CT * 128], BF16, tag="S", name="S")
        S3 = S_r.rearrange("p (t j) -> p t j", j=128)
        nc.vector.tensor_tensor(
            out=S3[:, :nt, :],
            in0=iota3[:, :nt, :],
            in1=dst_sb[:, o : o + nt].unsqueeze(2).to_broadcast([128, nt, 128]),
            op=mybir.AluOpType.is_equal,
        )
        for t8 in range(nt):
            BB, first, last = m.tl[ww][r0 + t8]
            if first:
                psB[BB] = pp.tile([128, D], F32, tag="ps", name="ps")
            nc.tensor.matmul(
                psB[BB][:, :],
                S3[:, t8, :],
                msgs3[:, t8, :],
                start=first,
                stop=last,
                skip_group_check=True,
            )
            if last:
                nc.vector.tensor_add(
                    out=cur3[:, BB, :], in0=cur3[:, BB, :], in1=psB.pop(BB)[:, :]
                )
    assert not psB


def _norm_acc(nc, m, layer, normp, cur_sb, src_acc, dst_acc, p_out):
    EPS = 1e-12
    NCH = 8
    cur3 = cur_sb.rearrange("p (b d) -> p b d", d=D)
    ss = normp.tile([128, m.NB], F32, tag="ss", name="ss")
    for b0 in range(0, m.NB, NCH):
        bl = min(NCH, m.NB - b0)
        sq = normp.tile([128, NCH * D], F32, tag="sq", name="sq")
        nc.vector.tensor_mul(
            out=sq[:, : bl * D],
            in0=cur_sb[:, b0 * D : (b0 + bl) * D],
            in1=cur_sb[:, b0 * D : (b0 + bl) * D],
        )
        nc.vector.tensor_reduce(
            out=ss[:, b0 : b0 + bl],
            in_=sq.rearrange("p (b d) -> p b d", d=D)[:, :bl, :],
            axis=mybir.AxisListType.X,
            op=mybir.AluOpType.add,
        )
    nrm = normp.tile([128, m.NB], F32, tag="nrm", name="nrm")
    nc.scalar.sqrt(out=nrm[:, :], in_=ss[:, :])
    nc.vector.tensor_scalar_max(nrm[:, :], nrm[:, :], EPS)
    rn = normp.tile([128, m.NB], F32, tag="rn", name="rn")
    nc.vector.reciprocal(out=rn[:, :], in_=nrm[:, :])
    for b0 in range(0, m.NB, NCH):
        bl = min(NCH, m.NB - b0)
        at = normp.tile([128, NCH * D], F32, tag="at", name="at")
        nc.sync.dma_start(out=at[:, : bl * D], in_=src_acc[:, b0 * D : (b0 + bl) * D])
        ctr = normp.tile([128, NCH * D], F32, tag="sq", name="ctr")
        nc.vector.tensor_tensor(
            out=ctr.rearrange("p (b d) -> p b d", d=D)[:, :bl, :],
            in0=cur3[:, b0 : b0 + bl, :],
            in1=rn[:, b0 : b0 + bl].unsqueeze(2).to_broadcast([128, bl, D]),
            op=mybir.AluOpType.mult,
        )
        nc.vector.tensor_add(
            out=at[:, : bl * D], in0=at[:, : bl * D], in1=ctr[:, : bl * D]
        )
        if layer == 1:
            nc.sync.dma_start(
                out=dst_acc[:, b0 * D : (b0 + bl) * D], in_=at[:, : bl * D]
            )
        else:
            nc.vector.tensor_scalar_mul(at[:, : bl * D], at[:, : bl * D], 1.0 / 3.0)
            nc.sync.dma_start(
                out=p_out[:, b0 * D : (b0 + bl) * D], in_=at[:, : bl * D]
            )


def _build_kernel(metas):
    nc = bacc.Bacc("TRN2", target_bir_lowering=False, debug=False, num_devices=NCORES,
                   num_swdge_queues=4)

    p_t0, p_sh0, p_idx, p_val, p_dst, p_out = {}, {}, {}, {}, {}, {}
    for g, m in metas.items():
        p_t0[g] = nc.declare_dram_parameter(f"t0_{g}", [m.npad, D], F32, False)
        p_sh0[g] = nc.declare_dram_parameter(f"shard0_{g}", [128, m.NB * D], F32, False)
        p_idx[g] = nc.declare_dram_parameter(f"idx_{g}", [128, 8 * m.Ttot], I16, False)
        p_val[g] = nc.declare_dram_parameter(f"val_{g}", [128, m.Ttot], F32, False)
        p_dst[g] = nc.declare_dram_parameter(f"dst_{g}", [128, m.Ttot], BF16, False)
    p_c = {
        nm: nc.declare_dram_parameter(nm, list(a.shape), BF16, False)
        for nm, a in _consts().items()
    }
    for g, m in metas.items():
        p_out[g] = nc.declare_dram_parameter(f"out_{g}", [128, m.NB * D], F32, True)

    accD = {g: nc.dram_tensor(f"accD_{g}", [128, m.NB * D], F32, kind="Internal")
            for g, m in metas.items()}
    t1loc = {g: nc.dram_tensor(f"t1loc_{g}", [m.nc_rows, D], F32, kind="Internal")
             for g, m in metas.items()}
    t1g = {g: nc.dram_tensor(f"t1g_{g}", [m.npad, D], F32, kind="Internal",
                             addr_space="Shared")
           for g, m in metas.items()}

    ALL = [list(range(NCORES))]
    gorder = list(metas.keys())

    with tile.TileContext(nc) as tc:
        with tc.tile_pool(name="constp", bufs=1) as constp, tc.tile_pool(
            name="curp", bufs=1
        ) as curp, tc.tile_pool(name="mTp", bufs=12) as mTp, tc.tile_pool(
            name="metap", bufs=12
        ) as metap, tc.tile_pool(name="msgp", bufs=4) as msgp, tc.tile_pool(
            name="sp", bufs=4
        ) as sp, tc.tile_pool(name="normp", bufs=1) as normp, tc.tile_pool(
            name="pp", bufs=4, space="PSUM"
        ) as pp:
            consts_sb = {}
            for nm, ph in p_c.items():
                t = constp.tile(list(ph.shape), BF16, name=nm)
                nc.sync.dma_start(out=t[:, :], in_=ph[:, :])
                consts_sb[nm] = t
            maxNBD = max(m.NB for m in metas.values()) * D

            pools = dict(mTp=mTp, metap=metap, msgp=msgp, sp=sp, pp=pp)

            def prep_t1(g, m, cur_sb):
                # cur_sb [p, (b f)] f32 -> t1loc rows r = b*128+p
                t1v = t1loc[g].rearrange("(b p) f -> p b f", p=128)
                nc.sync.dma_start(
                    out=t1v[:, :, :],
                    in_=cur_sb.rearrange("p (b f) -> p b f", f=D)[:, : m.NB, :],
                )

            def ag(g):
                nc.gpsimd.collective_compute(
                    "AllGather",
                    mybir.AluOpType.bypass,
                    ins=[t1loc[g][:, :].opt()],
                    outs=[t1g[g][:, :].opt()],
                    replica_groups=ALL,
                )

            cur_sb = curp.tile([128, maxNBD], F32, name="cur", tag="cur")
            for layer in (1, 2):
                for gi, g in enumerate(gorder):
                    m = metas[g]
                    src_tab = p_t0[g] if layer == 1 else t1g[g]
                    _spmm_layer(nc, m, g, layer, src_tab, p_idx[g], p_val[g],
                                p_dst[g], consts_sb, pools, cur_sb)
                    src_acc = p_sh0[g] if layer == 1 else accD[g]
                    if layer == 1:
                        prep_t1(g, m, cur_sb)
                        ag(g)
                    _norm_acc(nc, m, layer, normp, cur_sb, src_acc, accD[g],
                              p_out[g])

    nc.compile()
    return nc


def _run(graphs):
    """graphs: dict g -> (n, idx[2,E], val[E], table[n,D] f32). Returns aggs dict."""
    global LAST_EXEC_NS
    import os

    metas, tables = {}, {}
    for g, (n, idx, val, table) in graphs.items():
        idx = np.asarray(idx)
        metas[g] = GMeta(g, idx[0], idx[1], np.asarray(val), n)
        tables[g] = np.asarray(table, np.float32)

    nc = _build_kernel(metas)

    consts = _consts()
    ptabs = {g: m.ptable(tables[g]) for g, m in metas.items()}
    shards = {g: m.shard0(ptabs[g]) for g, m in metas.items()}
    in_maps = []
    for kk in range(NCORES):
        im = dict(consts)
        for g, m in metas.items():
            im[f"t0_{g}"] = ptabs[g]
            im[f"shard0_{g}"] = shards[g][kk]
            im[f"idx_{g}"] = m.idxw[kk]
            im[f"val_{g}"] = m.val[kk]
            im[f"dst_{g}"] = m.dst[kk]
        in_maps.append(im)

    trace = bool(os.environ.get("GNN_KERNEL_TRACE"))
    res = run_bass_kernel_spmd(nc, in_maps, core_ids=list(range(NCORES)), trace=trace)
    LAST_EXEC_NS = res.exec_time_ns

    aggs = {}
    for g, m in metas.items():
        outs = [res.results[k][f"out_{g}"] for k in range(NCORES)]
        aggs[g] = m.unshard(outs)
    return aggs


def kernel(users_feature, bundles_feature, items_feature,
           ub_idx, ub_val, ui_idx, ui_val, bi_idx, bi_val):
    graphs = {
        "ub": (NU + NBU, ub_idx, ub_val,
               np.concatenate([users_feature, bundles_feature])),
        "ui": (NU + NI_, ui_idx, ui_val,
               np.concatenate([users_feature, items_feature])),
        "bi": (NBU + NI_, bi_idx, bi_val,
               np.concatenate([bundles_feature, items_feature])),
    }
    aggs = _run(graphs)
    return np.ascontiguousarray(
        np.concatenate(
            [aggs["ub"][:NU], aggs["ub"][NU:],
             aggs["ui"][:NU], aggs["ui"][NU:],
             aggs["bi"][:NBU], aggs["bi"][NBU:]]
        ).astype(np.float32)
    )
